# revision 18
# baseline (speedup 1.0000x reference)
"""RNN-T JointNetwork kernel for 8 Trainium2 NeuronCores.

reference:
    combined = f[:, :, None, :] + p[:, None, :, :]   # (B,T,U,H)
    h = relu(combined)
    logits = einsum('btuh,vh->btuv', h, W) + b        # (B,T,U,V)

Shapes: f (8,256,640) p (8,64,640) W (1024,640) b (1024,) -> out (8,256,64,1024) f32.

Sharding: data-parallel over B — core i computes batch i. W/b replicated.

Per-core program (SPMD, bf16 matmuls):
  - inputs pre-transposed on host: ft=f[b].T (640,256) f32, pt=p[b].T (640,64) f32,
    wt=W.T (640,1024) bf16, bias replicated to (128,1024) f32.
  - h_u[h,t] = relu(ft[h,t] + pt[h,u]) via ScalarE activation (bias = pt column),
    written in bf16 (halves SBUF traffic; LDWEIGHTS gets the fast non-fp32 path).
  - logits[t, u, :] via PE: psum = h_u[kchunk, tslice].T @ wt[kchunk, vslice]
    accumulated over 5 k-chunks; DVE adds bias while copying PSUM->SBUF;
    each (t-tile, u) slice (512 KiB) is DMA'd out as soon as it is ready so
    the drain tail after the last matmul is only ~1 chunk deep.
  - a short burst of zero warmup matmuls at t=0 keeps the PE busy while the
    input DMAs land, so the HAM clock-gate is already at 8/8 (2.4 GHz) when
    the real matmul stream starts.
"""

import numpy as np
import ml_dtypes

import concourse.bass as bass
import concourse.mybir as mybir
import concourse.tile as tile
from concourse.bass_utils import run_bass_kernel_spmd
from concourse.vector_clock import ScopedClock

B, T, U, H, V = 8, 256, 64, 640, 1024
KC = H // 128          # 5 contraction chunks
TC = T // 128          # 2 t chunks
N_CORES = 8
MM_DT = mybir.dt.bfloat16
N_WARMUP_MM = 4

_PATCHED = False


_MAX_WAITS = 1  # this walrus build rejects >1 sem-wait per instruction


def _spill_waits(nc, inst, add):
    """If `inst` carries more than _MAX_WAITS sem-waits, move the excess onto
    same-engine nops emitted (in program order) just before it."""
    si = inst.sync_info
    waits = list(si.on_wait) if si and si.on_wait else []
    if len(waits) <= _MAX_WAITS:
        return
    excess = waits[: len(waits) - _MAX_WAITS]
    inst.sync_info = mybir.SyncInfo(
        on_wait=waits[len(waits) - _MAX_WAITS :],
        on_update=list(si.on_update or []),
    )
    for i in range(0, len(excess), _MAX_WAITS):
        nop = mybir.InstNoOp(name=f"{inst.name}_spillw{i}", ins=[], outs=[])
        nop.engine = inst.engine
        nop.sync_info = mybir.SyncInfo(
            on_wait=excess[i : i + _MAX_WAITS], on_update=[]
        )
        nc.register_instruction(nop, overwrite=True)
        add(nop)


def _patch_tile_drain():
    """This walrus build's setupSyncWait rejects instructions carrying more
    than one sem-wait.  Tile freely emits several per instruction, so (a)
    split excess waits onto same-engine nops as instructions are committed
    into basic blocks, and (b) do the same for the end-of-kernel drain."""
    global _PATCHED
    if _PATCHED:
        return
    _PATCHED = True

    orig_add = tile.TileContext._add_instruction

    def _add_instruction(self, inst):
        _spill_waits(self.nc, inst, lambda n: orig_add(self, n))
        orig_add(self, inst)

    tile.TileContext._add_instruction = _add_instruction

    def _drain_and_barrier(self, tick_clock, wait_clock):
        nc = self.nc
        probe = nc.sync.nop(nofuse=True, hint="drain_wait_probe")
        wait_clock.add_sem_waits(
            probe.ins, ScopedClock({None: tick_clock.global_clock})
        )
        si = probe.ins.sync_info
        waits = list(si.on_wait) if si and si.on_wait else []
        if len(waits) > _MAX_WAITS:
            probe.ins.sync_info = mybir.SyncInfo(
                on_wait=waits[:_MAX_WAITS], on_update=list(si.on_update or [])
            )
            rest = waits[_MAX_WAITS:]
            for i in range(0, len(rest), _MAX_WAITS):
                extra = nc.sync.nop(nofuse=True, hint=f"drain_wait_{i}")
                extra.ins.sync_info = mybir.SyncInfo(
                    on_wait=rest[i : i + _MAX_WAITS], on_update=[]
                )
        nc.sync.drain()
        nc.all_engine_barrier()
        assert self.sems is not None
        popped = nc._tile_sem_poison_stack.pop()
        assert popped is self._sem_poison
        nc.clear_and_free_semaphores(list(self.sems.allocated().values()))
        nc.all_engine_barrier()

    tile.TileContext._drain_and_barrier = _drain_and_barrier


def build_program():
    """One SPMD NeuronCore program: (T,U,V) joint-network slice for one batch."""
    _patch_tile_drain()
    nc = bass.Bass()
    f32 = mybir.dt.float32

    # ft/pt arrive partition-major ([128, KC*T] / [128, KC*U]) so each SBUF
    # partition is a single contiguous DMA descriptor.
    ft = nc.dram_tensor("ft", [128, KC * T], MM_DT, kind="ExternalInput")
    pt = nc.dram_tensor("pt", [128, KC * U], MM_DT, kind="ExternalInput")
    wt = nc.dram_tensor("wt", [H, V], MM_DT, kind="ExternalInput")
    bias = nc.dram_tensor("bias", [1, V], MM_DT, kind="ExternalInput")
    out = nc.dram_tensor("out", [T, U, V], f32, kind="ExternalOutput")

    ft_v = ft.rearrange("p (k t) -> p k t", k=KC)
    pt_v = pt.rearrange("p (k u) -> p k u", k=KC)
    wt_v = wt.rearrange("(k p) v -> p k v", p=128)

    with tile.TileContext(nc) as tc:
        with (
            tc.tile_pool(name="const", bufs=1) as cpool,
            tc.tile_pool(name="h", bufs=3) as hpool,
            tc.tile_pool(name="stage", bufs=6) as spool,
            tc.tile_pool(name="psum", bufs=8, space="PSUM") as ppool,
        ):
            # ── PE warmup: zero matmuls with no DMA deps keep the PE busy
            # while inputs stream in, so HAM reaches 8/8 before real work.
            warm_w = cpool.tile([128, 512], MM_DT, name="warm_w")
            nc.vector.memset(warm_w[:], 0)
            for i in range(N_WARMUP_MM):
                warm_ps = ppool.tile([128, 512], f32, tag="ps", name=f"warm{i}")
                nc.tensor.matmul(
                    warm_ps[:], warm_w[:, :128], warm_w[:],
                    start=True, stop=True,
                )
            # Dummy activation with no DMA deps: pulls the lazy
            # ACT_TABLE_LOAD (~1.3us) off the first real activation's
            # critical path into the startup window.
            warm_act = cpool.tile([128, 1], f32, name="warm_act")
            nc.vector.memset(warm_act[:], 0)
            nc.scalar.activation(
                warm_act[:], warm_act[:], mybir.ActivationFunctionType.Relu
            )

            # Input DMA order feeds the pipeline front-to-back: ft+pt (bf16,
            # unblocks the ScalarE h chain) before the five weight chunks.
            # The head is input-bandwidth-bound (~1.7 MB), so bf16 f/p halves
            # the time until the first full k-group can stream.
            ft_sb = cpool.tile([128, KC, T], MM_DT)
            pt_sb = cpool.tile([128, KC, U], MM_DT)
            wt_ks = [cpool.tile([128, V], MM_DT, name=f"wt_k{k}")
                     for k in range(KC)]
            bias_row = cpool.tile([1, V], MM_DT, name="bias_row")
            bias_sb = cpool.tile([128, V], f32)
            # bias row (2 KiB) rides the otherwise-idle GpSimd queue so it
            # costs no issue slot on the Sync queue, whose last wt chunk
            # gates the start of full-rate streaming.
            nc.gpsimd.dma_start(bias_row[:], bias[:])
            nc.sync.dma_start(ft_sb[:], ft_v[:])
            nc.sync.dma_start(pt_sb[:], pt_v[:])
            for k in range(KC):
                nc.sync.dma_start(wt_ks[k][:], wt_v[:, k, :])

            # Broadcast bias (1,V) -> (128,V) on chip: ones(1,128).T @ bias
            # via two K=1 matmuls + DVE copies, instead of shipping a 512 KiB
            # replicated tensor over the already-saturated input DMA window.
            ones_t = cpool.tile([1, 128], MM_DT, name="ones_t")
            nc.vector.memset(ones_t[:], 1.0)
            for h_ in range(2):
                sl = slice(h_ * 512, (h_ + 1) * 512)
                bps = ppool.tile([128, 512], f32, tag="ps", name=f"bias_ps{h_}")
                nc.tensor.matmul(bps[:], ones_t[:], bias_row[:, sl],
                                 start=True, stop=True)
                nc.vector.tensor_copy(bias_sb[:, sl], bps[:])

            for u in range(U):
                h_u = hpool.tile([128, KC, T], MM_DT, tag="h")
                for k in range(KC):
                    nc.scalar.activation(
                        h_u[:, k, :],
                        ft_sb[:, k, :],
                        mybir.ActivationFunctionType.Relu,
                        bias=pt_sb[:, k, u : u + 1],
                    )
                for t_ in range(TC):
                    st = spool.tile([128, V], f32, tag="st", name=f"st{u}_{t_}")
                    psums = [ppool.tile([128, 512], f32, tag="ps",
                                        name=f"ps{u}_{t_}_{h_}")
                             for h_ in range(2)]
                    for k in range(KC):
                        lhsT = h_u[:, k, t_ * 128 : (t_ + 1) * 128]
                        for h_ in range(2):
                            nc.tensor.matmul(
                                psums[h_][:],
                                lhsT,
                                wt_ks[k][:, h_ * 512 : (h_ + 1) * 512],
                                start=(k == 0),
                                stop=(k == KC - 1),
                            )
                    for h_ in range(2):
                        sl = slice(h_ * 512, (h_ + 1) * 512)
                        nc.vector.tensor_add(st[:, sl], psums[h_][:], bias_sb[:, sl])
                        if u == U - 1:
                            # last u: write each v-half as soon as its bias
                            # add lands, halving the end-of-kernel DMA drain
                            nc.sync.dma_start(
                                out[t_ * 128 : (t_ + 1) * 128, u, sl],
                                st[:, sl],
                            )
                    if u < U - 1:
                        nc.sync.dma_start(
                            out[t_ * 128 : (t_ + 1) * 128, u, :], st[:]
                        )
    return nc


def make_in_maps(f, p, W, b):
    """Per-core input dict for run_bass_kernel_spmd (core i <- batch i)."""
    f = np.asarray(f, np.float32)
    p = np.asarray(p, np.float32)
    W = np.asarray(W, np.float32)
    b = np.asarray(b, np.float32)
    wt = np.ascontiguousarray(W.T).astype(ml_dtypes.bfloat16)   # (H, V) bf16
    bias = np.ascontiguousarray(b[None, :]).astype(ml_dtypes.bfloat16)

    def pmajor(x, n):  # (H, n) -> (128, KC*n): partition-major, k consecutive
        return np.ascontiguousarray(
            x.reshape(KC, 128, n).transpose(1, 0, 2).reshape(128, KC * n)
        ).astype(ml_dtypes.bfloat16)

    return [
        {
            "ft": pmajor(f[i].T, T),
            "pt": pmajor(p[i].T, U),
            "wt": wt,
            "bias": bias,
        }
        for i in range(N_CORES)
    ]


def kernel(f, p, W, b):
    nc = build_program()
    in_maps = make_in_maps(f, p, W, b)
    res = run_bass_kernel_spmd(nc, in_maps, list(range(N_CORES)))
    return np.stack([res.results[i]["out"] for i in range(N_CORES)], axis=0)


# revision 19
# speedup vs baseline: 1.0025x; 1.0025x over previous
"""RNN-T JointNetwork kernel for 8 Trainium2 NeuronCores.

reference:
    combined = f[:, :, None, :] + p[:, None, :, :]   # (B,T,U,H)
    h = relu(combined)
    logits = einsum('btuh,vh->btuv', h, W) + b        # (B,T,U,V)

Shapes: f (8,256,640) p (8,64,640) W (1024,640) b (1024,) -> out (8,256,64,1024) f32.

Sharding: data-parallel over B — core i computes batch i. W/b replicated.

Per-core program (SPMD, bf16 matmuls):
  - inputs pre-transposed on host: ft=f[b].T (640,256) f32, pt=p[b].T (640,64) f32,
    wt=W.T (640,1024) bf16, bias replicated to (128,1024) f32.
  - h_u[h,t] = relu(ft[h,t] + pt[h,u]) via ScalarE activation (bias = pt column),
    written in bf16 (halves SBUF traffic; LDWEIGHTS gets the fast non-fp32 path).
  - logits[t, u, :] via PE: psum = h_u[kchunk, tslice].T @ wt[kchunk, vslice]
    accumulated over 5 k-chunks; DVE adds bias while copying PSUM->SBUF;
    each (t-tile, u) slice (512 KiB) is DMA'd out as soon as it is ready so
    the drain tail after the last matmul is only ~1 chunk deep.
  - a short burst of zero warmup matmuls at t=0 keeps the PE busy while the
    input DMAs land, so the HAM clock-gate is already at 8/8 (2.4 GHz) when
    the real matmul stream starts.
"""

import numpy as np
import ml_dtypes

import concourse.bass as bass
import concourse.mybir as mybir
import concourse.tile as tile
from concourse.bass_utils import run_bass_kernel_spmd
from concourse.vector_clock import ScopedClock

B, T, U, H, V = 8, 256, 64, 640, 1024
KC = H // 128          # 5 contraction chunks
TC = T // 128          # 2 t chunks
N_CORES = 8
MM_DT = mybir.dt.bfloat16
N_WARMUP_MM = 18

_PATCHED = False


_MAX_WAITS = 1  # this walrus build rejects >1 sem-wait per instruction


def _spill_waits(nc, inst, add):
    """If `inst` carries more than _MAX_WAITS sem-waits, move the excess onto
    same-engine nops emitted (in program order) just before it."""
    si = inst.sync_info
    waits = list(si.on_wait) if si and si.on_wait else []
    if len(waits) <= _MAX_WAITS:
        return
    excess = waits[: len(waits) - _MAX_WAITS]
    inst.sync_info = mybir.SyncInfo(
        on_wait=waits[len(waits) - _MAX_WAITS :],
        on_update=list(si.on_update or []),
    )
    for i in range(0, len(excess), _MAX_WAITS):
        nop = mybir.InstNoOp(name=f"{inst.name}_spillw{i}", ins=[], outs=[])
        nop.engine = inst.engine
        nop.sync_info = mybir.SyncInfo(
            on_wait=excess[i : i + _MAX_WAITS], on_update=[]
        )
        nc.register_instruction(nop, overwrite=True)
        add(nop)


def _patch_tile_drain():
    """This walrus build's setupSyncWait rejects instructions carrying more
    than one sem-wait.  Tile freely emits several per instruction, so (a)
    split excess waits onto same-engine nops as instructions are committed
    into basic blocks, and (b) do the same for the end-of-kernel drain."""
    global _PATCHED
    if _PATCHED:
        return
    _PATCHED = True

    orig_add = tile.TileContext._add_instruction

    def _add_instruction(self, inst):
        _spill_waits(self.nc, inst, lambda n: orig_add(self, n))
        orig_add(self, inst)

    tile.TileContext._add_instruction = _add_instruction

    def _drain_and_barrier(self, tick_clock, wait_clock):
        nc = self.nc
        probe = nc.sync.nop(nofuse=True, hint="drain_wait_probe")
        wait_clock.add_sem_waits(
            probe.ins, ScopedClock({None: tick_clock.global_clock})
        )
        si = probe.ins.sync_info
        waits = list(si.on_wait) if si and si.on_wait else []
        if len(waits) > _MAX_WAITS:
            probe.ins.sync_info = mybir.SyncInfo(
                on_wait=waits[:_MAX_WAITS], on_update=list(si.on_update or [])
            )
            rest = waits[_MAX_WAITS:]
            for i in range(0, len(rest), _MAX_WAITS):
                extra = nc.sync.nop(nofuse=True, hint=f"drain_wait_{i}")
                extra.ins.sync_info = mybir.SyncInfo(
                    on_wait=rest[i : i + _MAX_WAITS], on_update=[]
                )
        nc.sync.drain()
        nc.all_engine_barrier()
        assert self.sems is not None
        popped = nc._tile_sem_poison_stack.pop()
        assert popped is self._sem_poison
        nc.clear_and_free_semaphores(list(self.sems.allocated().values()))
        nc.all_engine_barrier()

    tile.TileContext._drain_and_barrier = _drain_and_barrier


def build_program():
    """One SPMD NeuronCore program: (T,U,V) joint-network slice for one batch."""
    _patch_tile_drain()
    nc = bass.Bass()
    f32 = mybir.dt.float32

    # ft/pt arrive partition-major ([128, KC*T] / [128, KC*U]) so each SBUF
    # partition is a single contiguous DMA descriptor.
    ft = nc.dram_tensor("ft", [128, KC * T], MM_DT, kind="ExternalInput")
    pt = nc.dram_tensor("pt", [128, KC * U], MM_DT, kind="ExternalInput")
    wt = nc.dram_tensor("wt", [H, V], MM_DT, kind="ExternalInput")
    bias = nc.dram_tensor("bias", [1, V], MM_DT, kind="ExternalInput")
    out = nc.dram_tensor("out", [T, U, V], f32, kind="ExternalOutput")

    ft_v = ft.rearrange("p (k t) -> p k t", k=KC)
    pt_v = pt.rearrange("p (k u) -> p k u", k=KC)
    wt_v = wt.rearrange("(k p) v -> p k v", p=128)

    with tile.TileContext(nc) as tc:
        with (
            tc.tile_pool(name="const", bufs=1) as cpool,
            tc.tile_pool(name="h", bufs=3) as hpool,
            tc.tile_pool(name="stage", bufs=6) as spool,
            tc.tile_pool(name="psum", bufs=8, space="PSUM") as ppool,
        ):
            # ── PE warmup: zero matmuls with no DMA deps keep the PE busy
            # while inputs stream in, so HAM reaches 8/8 before real work.
            warm_w = cpool.tile([128, 512], MM_DT, name="warm_w")
            nc.vector.memset(warm_w[:], 0)
            for i in range(N_WARMUP_MM):
                warm_ps = ppool.tile([128, 512], f32, tag="ps", name=f"warm{i}")
                nc.tensor.matmul(
                    warm_ps[:], warm_w[:, :128], warm_w[:],
                    start=True, stop=True,
                )
            # Dummy activation with no DMA deps: pulls the lazy
            # ACT_TABLE_LOAD (~1.3us) off the first real activation's
            # critical path into the startup window.
            warm_act = cpool.tile([128, 1], f32, name="warm_act")
            nc.vector.memset(warm_act[:], 0)
            nc.scalar.activation(
                warm_act[:], warm_act[:], mybir.ActivationFunctionType.Relu
            )

            # Input DMA order feeds the pipeline front-to-back: ft+pt (bf16,
            # unblocks the ScalarE h chain) before the five weight chunks.
            # The head is input-bandwidth-bound (~1.7 MB), so bf16 f/p halves
            # the time until the first full k-group can stream.
            ft_sb = cpool.tile([128, KC, T], MM_DT)
            pt_sb = cpool.tile([128, KC, U], MM_DT)
            wt_ks = [cpool.tile([128, V], MM_DT, name=f"wt_k{k}")
                     for k in range(KC)]
            bias_row = cpool.tile([1, V], MM_DT, name="bias_row")
            bias_sb = cpool.tile([128, V], f32)
            # bias row (2 KiB) rides the otherwise-idle GpSimd queue so it
            # costs no issue slot on the Sync queue, whose last wt chunk
            # gates the start of full-rate streaming.
            nc.gpsimd.dma_start(bias_row[:], bias[:])
            nc.sync.dma_start(ft_sb[:], ft_v[:])
            nc.sync.dma_start(pt_sb[:], pt_v[:])
            for k in range(KC):
                nc.sync.dma_start(wt_ks[k][:], wt_v[:, k, :])

            # Broadcast bias (1,V) -> (128,V) on chip: ones(1,128).T @ bias
            # via two K=1 matmuls + DVE copies, instead of shipping a 512 KiB
            # replicated tensor over the already-saturated input DMA window.
            ones_t = cpool.tile([1, 128], MM_DT, name="ones_t")
            nc.vector.memset(ones_t[:], 1.0)
            for h_ in range(2):
                sl = slice(h_ * 512, (h_ + 1) * 512)
                bps = ppool.tile([128, 512], f32, tag="ps", name=f"bias_ps{h_}")
                nc.tensor.matmul(bps[:], ones_t[:], bias_row[:, sl],
                                 start=True, stop=True)
                nc.vector.tensor_copy(bias_sb[:, sl], bps[:])

            for u in range(U):
                h_u = hpool.tile([128, KC, T], MM_DT, tag="h")
                for k in range(KC):
                    nc.scalar.activation(
                        h_u[:, k, :],
                        ft_sb[:, k, :],
                        mybir.ActivationFunctionType.Relu,
                        bias=pt_sb[:, k, u : u + 1],
                    )
                for t_ in range(TC):
                    st = spool.tile([128, V], f32, tag="st", name=f"st{u}_{t_}")
                    psums = [ppool.tile([128, 512], f32, tag="ps",
                                        name=f"ps{u}_{t_}_{h_}")
                             for h_ in range(2)]
                    for k in range(KC):
                        lhsT = h_u[:, k, t_ * 128 : (t_ + 1) * 128]
                        for h_ in range(2):
                            nc.tensor.matmul(
                                psums[h_][:],
                                lhsT,
                                wt_ks[k][:, h_ * 512 : (h_ + 1) * 512],
                                start=(k == 0),
                                stop=(k == KC - 1),
                            )
                    for h_ in range(2):
                        sl = slice(h_ * 512, (h_ + 1) * 512)
                        nc.vector.tensor_add(st[:, sl], psums[h_][:], bias_sb[:, sl])
                        if u == U - 1:
                            # last u: write each v-half as soon as its bias
                            # add lands, halving the end-of-kernel DMA drain
                            nc.sync.dma_start(
                                out[t_ * 128 : (t_ + 1) * 128, u, sl],
                                st[:, sl],
                            )
                    if u < U - 1:
                        nc.sync.dma_start(
                            out[t_ * 128 : (t_ + 1) * 128, u, :], st[:]
                        )
    return nc


def make_in_maps(f, p, W, b):
    """Per-core input dict for run_bass_kernel_spmd (core i <- batch i)."""
    f = np.asarray(f, np.float32)
    p = np.asarray(p, np.float32)
    W = np.asarray(W, np.float32)
    b = np.asarray(b, np.float32)
    wt = np.ascontiguousarray(W.T).astype(ml_dtypes.bfloat16)   # (H, V) bf16
    bias = np.ascontiguousarray(b[None, :]).astype(ml_dtypes.bfloat16)

    def pmajor(x, n):  # (H, n) -> (128, KC*n): partition-major, k consecutive
        return np.ascontiguousarray(
            x.reshape(KC, 128, n).transpose(1, 0, 2).reshape(128, KC * n)
        ).astype(ml_dtypes.bfloat16)

    return [
        {
            "ft": pmajor(f[i].T, T),
            "pt": pmajor(p[i].T, U),
            "wt": wt,
            "bias": bias,
        }
        for i in range(N_CORES)
    ]


def kernel(f, p, W, b):
    nc = build_program()
    in_maps = make_in_maps(f, p, W, b)
    res = run_bass_kernel_spmd(nc, in_maps, list(range(N_CORES)))
    return np.stack([res.results[i]["out"] for i in range(N_CORES)], axis=0)
